# revision 9
# baseline (speedup 1.0000x reference)
"""Bass/Trainium2 kernel for nn_EquivariantReynoldsWrap.

The reference module is linear in x: for every pixel,
    out = (1/G) * sum_g BlockDiag(A_g) @ Wf @ BlockDiag(Ainv_g) @ x_pixel
so the whole pipeline collapses into one 64x64 channel-mixing matrix M,
computed on host (cheap). The device work is a single 1x1-conv matmul
out[b] = M @ x[b] with x[b] viewed as (64, H*W).

Sharding: data-parallel over B across the 8 cores (1 batch each).
Per core the two halves of the pixel axis are interleaved on the
partition axis (partition p = channel p//2, half p%2) and the stationary
weight is the 128x128 interleaved block-diagonal of M^T, so each
512-column matmul covers 1024 pixels.

v3: bf16 end-to-end; host pre-packs the exact SBUF images so each HWDGE
ring does ONE fully-contiguous transfer (per-transfer ring setup is
~1.4-1.6us, so fewer/bigger transfers win):
  sync   ring: [W | x_lo]   scalar ring: [x_hi]
PSUM->SBUF copies run on DVE + GpSimd (no scalar ACTIVATE -> no
ACT_TABLE_LOAD occupying the scalar ring ahead of x). Output is written
bf16 and upcast on host. 7x512 bf16 warmups un-throttle HAM before the
data lands (the tensor sequencer's speed also drives the fixed NEFF
epilogue's semaphore-zeroing, which is on the critical path).

Raw bacc (no TileContext): hand-rolled semaphores, minimal head/tail.
"""

import numpy as np
import ml_dtypes

import concourse.bacc as bacc
import concourse.bass as bass
from concourse import mybir
from concourse.bass_utils import run_bass_kernel_spmd

B, C, H, W_SP = 8, 64, 64, 64
COUT = 64
HW = H * W_SP          # 4096 pixels per batch
HALF = HW // 2         # 2048 -> stacked column count per core
N_CORES = 8

N_WARM = 6             # bf16 warm-up matmuls (HAM un-throttle)
WARM_COLS = 512

TRACE = False          # test.py flips this to profile
_cached_nc = None

BF16 = ml_dtypes.bfloat16


def _build_nc():
    global _cached_nc
    if _cached_nc is not None:
        return _cached_nc

    bf16 = mybir.dt.bfloat16
    f32 = mybir.dt.float32

    nc = bacc.Bacc(
        "TRN2",
        target_bir_lowering=False,
        debug=False,
        enable_asserts=False,
        num_devices=N_CORES,
    )
    # xb = [W | x_lo]: 128 cols of weights then 1024 cols of x (pixel
    # cols 0..1023), on the sync ring. xa = x_hi (pixel cols 1024..2047)
    # on the scalar ring. Both are exact SBUF images -> one
    # fully-contiguous DMA per ring.
    xbd = nc.dram_tensor("xb", [128, 128 + 1024], bf16, kind="ExternalInput").ap()
    xad = nc.dram_tensor("xa", [128, 1024], bf16, kind="ExternalInput").ap()
    yd = nc.dram_tensor("y", [128, HALF], bf16, kind="ExternalOutput").ap()

    with (
        nc.sbuf_tensor("xb_t", [128, 128 + 1024], bf16) as xb_t,
        nc.sbuf_tensor("xa_t", [128, 1024], bf16) as xa_t,
        nc.sbuf_tensor("ot", [128, HALF], bf16) as ot_t,
        nc.sbuf_tensor("zt", [128, WARM_COLS], bf16) as zt_t,
        nc.psum_tensor([128, HALF], f32) as ps_t,
        nc.psum_tensor([128, 512], f32) as wps_t,
        nc.semaphore("s_xb") as s_xb,    # sync ring: w + x_lo landed
        nc.semaphore("s_xa") as s_xa,    # scalar ring: x_hi landed
        nc.semaphore("s_z") as s_z,      # warmup tile zeroed
        nc.semaphore("s_mm") as s_mm,    # matmul per chunk
        nc.semaphore("s_cpv") as s_cpv,  # DVE copies (c0, c2)
        nc.semaphore("s_cpg") as s_cpg,  # GpSimd copies (c1, c3)
        nc.semaphore("s_y") as s_y,      # out DMAs
    ):
        xb = xb_t.ap()
        xa = xa_t.ap()
        ot = ot_t.ap()
        zt = zt_t.ap()
        ps = ps_t.ap()
        wps = wps_t.ap()

        wt = xb[:, 0:128]          # stationary weights live inside xb

        # Linear emission into the entry basic block (no nc.Block): avoids
        # the per-engine body branches (I$ misses) and the Block exit
        # barrier; the walrus-generated NEFF epilogue handles quiescence
        # and zeroes all semaphores for re-execution.
        sync, scalar, tensor, vector, gpsimd = (
            nc.sync, nc.scalar, nc.tensor, nc.vector, nc.gpsimd
        )

        gpsimd.memset(zt[:], 0.0).then_inc(s_z)

        # one transfer per HWDGE ring, issued back-to-back at kernel start
        sync.dma_start(xb_t.ap(), xbd[:]).then_inc(s_xb, 16)
        scalar.dma_start(xa_t.ap(), xad[:]).then_inc(s_xa, 16)

        # HAM warm-up on zeroed bf16 tile (1 HW pass each); they overlap
        # the x DMA latency and should retire just before the data lands
        tensor.wait_ge(s_z, 1)
        for _ in range(N_WARM):
            tensor.matmul(wps[:], zt[:, :128], zt[:])

        # A matmul's sem update fires at instruction retire (last column
        # ENTERS the array); the ~128-cycle systolic drain is still
        # writing PSUM then. Chunk i's drain is covered by chunk i+1's
        # matmul (s_mm >= i+2); only the last chunk needs an explicit
        # short guard matmul to carry its inc.
        tensor.wait_ge(s_xb, 16)
        tensor.matmul(ps[:, 0:512], wt, xb[:, 128:640]).then_inc(s_mm)
        tensor.matmul(ps[:, 512:1024], wt, xb[:, 640:1152]).then_inc(s_mm)
        tensor.wait_ge(s_xa, 16)
        tensor.matmul(ps[:, 1024:1536], wt, xa[:, 0:512]).then_inc(s_mm)
        tensor.matmul(ps[:, 1536:2048], wt, xa[:, 512:1024]).then_inc(s_mm)
        tensor.matmul(wps[:, :128], zt[:, :128], zt[:, :128]).then_inc(s_mm)

        # copies (f32 psum -> bf16 sbuf): DVE takes c0/c2, ACT c1/c3.
        # The out-DMA triggers are sequencer-class ops, so each gates on
        # the copies' completion sems (queue order alone does NOT order
        # it after the datapath).
        vector.wait_ge(s_mm, 2)
        vector.tensor_copy(ot[:, 0:512], ps[:, 0:512]).then_inc(s_cpv)
        vector.wait_ge(s_mm, 4)
        vector.tensor_copy(ot[:, 1024:1536], ps[:, 1024:1536]).then_inc(s_cpv)

        scalar.wait_ge(s_mm, 3)
        scalar.copy(ot[:, 512:1024], ps[:, 512:1024]).then_inc(s_cpg)
        scalar.wait_ge(s_mm, 5)
        scalar.copy(ot[:, 1536:2048], ps[:, 1536:2048]).then_inc(s_cpg)

        # y triggers run in PARALLEL on the two HWDGE engines (the
        # epilogue entry barrier waits for the LAST engine, so serial
        # triggers on one engine would push it out by a full trigger).
        scalar.wait_ge(s_cpv, 1)
        scalar.wait_ge(s_cpg, 1)
        scalar.dma_start(yd[:, 0:1024], ot[:, 0:1024]).then_inc(s_y, 16)
        sync.wait_ge(s_cpv, 2)
        sync.wait_ge(s_cpg, 2)
        sync.dma_start(yd[:, 1024:2048], ot[:, 1024:2048]).then_inc(s_y, 16)
        # the NEFF epilogue's per-ring DGE drains hold teardown until all
        # output descriptors (data + sem incs) have retired
        _ = s_y

    nc.compile()
    _cached_nc = nc
    return nc


def _fuse_weights(group_tensor, group_tensor_inv, Wf):
    A = np.asarray(group_tensor, np.float64)
    Ai = np.asarray(group_tensor_inv, np.float64)
    Wf64 = np.asarray(Wf, np.float64)
    G, CG, _ = A.shape
    n = C // CG
    eye = np.eye(n)
    M = np.zeros((COUT, C))
    for g in range(G):
        M += np.kron(eye, A[g]) @ Wf64 @ np.kron(eye, Ai[g])
    M /= G
    MT = np.ascontiguousarray(M.T).astype(np.float32)
    # interleaved packing: x-tile partition p holds channel p//2 of pixel
    # half p%2; out partition q holds channel q//2 of half q%2.
    W2T = np.zeros((128, 128), np.float32)
    W2T[0::2, 0::2] = MT
    W2T[1::2, 1::2] = MT
    return W2T.astype(BF16)


def kernel(x, group_tensor, group_tensor_inv, Wf):
    nc = _build_nc()
    W2T = _fuse_weights(group_tensor, group_tensor_inv, Wf)
    x = np.asarray(x, np.float32)

    # partition p = channel p//2, pixel-half p%2: just a reshape of (C, HW)
    xr = x.reshape(B, 128, HALF).astype(BF16)
    in_maps = []
    for b in range(B):
        xb = np.empty((128, 128 + 1024), BF16)
        xb[:, 0:128] = W2T
        xb[:, 128:1152] = xr[b, :, 0:1024]
        xa = np.ascontiguousarray(xr[b, :, 1024:2048])
        in_maps.append({"xb": xb, "xa": xa})

    res = run_bass_kernel_spmd(
        nc, in_maps, core_ids=list(range(N_CORES)), trace=TRACE
    )
    if TRACE:
        kernel.last_results = res
    y = np.stack(
        [
            res.results[b]["y"].astype(np.float32).reshape(COUT, H, W_SP)
            for b in range(B)
        ]
    )
    return y


# revision 11
# speedup vs baseline: 1.1018x; 1.1018x over previous
"""Bass/Trainium2 kernel for nn_EquivariantReynoldsWrap.

The reference module is linear in x: for every pixel,
    out = (1/G) * sum_g BlockDiag(A_g) @ Wf @ BlockDiag(Ainv_g) @ x_pixel
so the whole pipeline collapses into one 64x64 channel-mixing matrix M,
computed on host (cheap). The device work is a single 1x1-conv matmul
out[b] = M @ x[b] with x[b] viewed as (64, H*W).

Sharding: data-parallel over B across the 8 cores (1 batch each).
Per core the two halves of the pixel axis are interleaved on the
partition axis (partition p = channel p//2, half p%2) and the stationary
weight is the 128x128 interleaved block-diagonal of M^T, so each
512-column matmul covers 1024 pixels.

v6: bf16 end-to-end; host pre-packs ONE exact SBUF image [W | x] so the
sync ring does a SINGLE fully-contiguous 557KB transfer (the HWDGE
descriptor path serializes across rings at ~300GB/s and each transfer
pays ~1.45us trigger->first-packet latency, so one big transfer beats
any split). Output is written bf16 and upcast on host. The y triggers
run in parallel: y_lo on the scalar engine (whose ring is otherwise
idle), y_hi on sync. 6x512 bf16 warmups ramp HAM before data lands.

Raw bacc (no TileContext): hand-rolled semaphores, minimal head/tail.
The walrus NEFF epilogue (global barrier + 253-semaphore zeroing, ~6.4us
with the Tensor sequencer on the critical path) is fixed overhead; the
kernel minimizes the time at which the LAST engine reaches the barrier.
"""

import numpy as np
import ml_dtypes

import concourse.bacc as bacc
import concourse.bass as bass
from concourse import mybir
from concourse.bass_utils import run_bass_kernel_spmd

B, C, H, W_SP = 8, 64, 64, 64
COUT = 64
HW = H * W_SP          # 4096 pixels per batch
HALF = HW // 2         # 2048 -> stacked column count per core
N_CORES = 8

N_WARM = 6             # bf16 warm-up matmuls (HAM un-throttle)
WARM_COLS = 512

TRACE = False          # test.py flips this to profile
_cached_nc = None

BF16 = ml_dtypes.bfloat16


def _build_nc():
    global _cached_nc
    if _cached_nc is not None:
        return _cached_nc

    bf16 = mybir.dt.bfloat16
    f32 = mybir.dt.float32

    nc = bacc.Bacc(
        "TRN2",
        target_bir_lowering=False,
        debug=False,
        enable_asserts=False,
        num_devices=N_CORES,
    )
    # xw = [W | x]: 128 cols of weights then 2048 cols of x; one exact
    # SBUF image -> a single fully-contiguous DMA on the sync ring.
    xwd = nc.dram_tensor("xw", [128, 128 + HALF], bf16, kind="ExternalInput").ap()
    yd = nc.dram_tensor("y", [128, HALF], bf16, kind="ExternalOutput").ap()

    with (
        nc.sbuf_tensor("xw_t", [128, 128 + HALF], bf16) as xw_t,
        nc.sbuf_tensor("ot", [128, HALF], bf16) as ot_t,
        nc.sbuf_tensor("zt", [128, WARM_COLS], bf16) as zt_t,
        nc.psum_tensor([128, HALF], f32) as ps_t,
        nc.psum_tensor([128, 512], f32) as wps_t,
        nc.semaphore("s_x") as s_x,      # sync ring: w + x landed
        nc.semaphore("s_z") as s_z,      # warmup tile zeroed
        nc.semaphore("s_mm") as s_mm,    # matmul per chunk
        nc.semaphore("s_cpv") as s_cpv,  # DVE copies (c0, c2)
        nc.semaphore("s_cpa") as s_cpa,  # ACT copies (c1, c3)
        nc.semaphore("s_y") as s_y,      # out DMAs
    ):
        xw = xw_t.ap()
        ot = ot_t.ap()
        zt = zt_t.ap()
        ps = ps_t.ap()
        wps = wps_t.ap()

        wt = xw[:, 0:128]          # stationary weights live inside xw

        # Linear emission into the entry basic block (no nc.Block): avoids
        # the per-engine body branches (I$ misses) and the Block exit
        # barrier; the walrus-generated NEFF epilogue handles quiescence
        # and zeroes all semaphores for re-execution.
        sync, scalar, tensor, vector, gpsimd = (
            nc.sync, nc.scalar, nc.tensor, nc.vector, nc.gpsimd
        )

        gpsimd.memset(zt[:], 0.0).then_inc(s_z)

        # single input transfer, issued at kernel start
        sync.dma_start(xw_t.ap(), xwd[:]).then_inc(s_x, 16)

        # HAM warm-up on zeroed bf16 tile (1 HW pass each); they overlap
        # the x DMA latency and retire just before the data lands
        tensor.wait_ge(s_z, 1)
        for _ in range(N_WARM):
            tensor.matmul(wps[:], zt[:, :128], zt[:])

        # A matmul's sem update fires at instruction retire (last column
        # ENTERS the array); the ~128-cycle systolic drain is still
        # writing PSUM then. Chunk i's drain is covered by chunk i+1's
        # matmul (s_mm >= i+2); only the last chunk needs an explicit
        # short guard matmul to carry its inc.
        tensor.wait_ge(s_x, 16)
        tensor.matmul(ps[:, 0:512], wt, xw[:, 128:640]).then_inc(s_mm)
        tensor.matmul(ps[:, 512:1024], wt, xw[:, 640:1152]).then_inc(s_mm)
        tensor.matmul(ps[:, 1024:1536], wt, xw[:, 1152:1664]).then_inc(s_mm)
        tensor.matmul(ps[:, 1536:2048], wt, xw[:, 1664:2176]).then_inc(s_mm)
        tensor.matmul(wps[:, :128], zt[:, :128], zt[:, :128]).then_inc(s_mm)

        # copies (f32 psum -> bf16 sbuf): DVE takes c0/c2, ACT c1/c3.
        # The out-DMA triggers are sequencer-class ops, so each gates on
        # the copies' completion sems (queue order alone does NOT order
        # it after the datapath).
        vector.wait_ge(s_mm, 2)
        vector.tensor_copy(ot[:, 0:512], ps[:, 0:512]).then_inc(s_cpv)
        vector.wait_ge(s_mm, 4)
        vector.tensor_copy(ot[:, 1024:1536], ps[:, 1024:1536]).then_inc(s_cpv)

        scalar.wait_ge(s_mm, 3)
        scalar.copy(ot[:, 512:1024], ps[:, 512:1024]).then_inc(s_cpa)
        scalar.wait_ge(s_mm, 5)
        scalar.copy(ot[:, 1536:2048], ps[:, 1536:2048]).then_inc(s_cpa)

        # y triggers run in PARALLEL on the two HWDGE engines (the
        # epilogue entry barrier waits for the LAST engine, so serial
        # triggers on one engine would push it out by a full trigger).
        # y_lo rides the otherwise-idle scalar ring.
        scalar.wait_ge(s_cpv, 1)
        scalar.wait_ge(s_cpa, 1)
        scalar.dma_start(yd[:, 0:1024], ot[:, 0:1024]).then_inc(s_y, 16)
        sync.wait_ge(s_cpv, 2)
        sync.wait_ge(s_cpa, 2)
        sync.dma_start(yd[:, 1024:2048], ot[:, 1024:2048]).then_inc(s_y, 16)
        # the NEFF epilogue's per-ring DGE drains hold teardown until all
        # output descriptors (data + sem incs) have retired
        _ = s_y

    nc.compile()
    _cached_nc = nc
    return nc


def _fuse_weights(group_tensor, group_tensor_inv, Wf):
    A = np.asarray(group_tensor, np.float64)
    Ai = np.asarray(group_tensor_inv, np.float64)
    Wf64 = np.asarray(Wf, np.float64)
    G, CG, _ = A.shape
    n = C // CG
    eye = np.eye(n)
    M = np.zeros((COUT, C))
    for g in range(G):
        M += np.kron(eye, A[g]) @ Wf64 @ np.kron(eye, Ai[g])
    M /= G
    MT = np.ascontiguousarray(M.T).astype(np.float32)
    # interleaved packing: x-tile partition p holds channel p//2 of pixel
    # half p%2; out partition q holds channel q//2 of half q%2.
    W2T = np.zeros((128, 128), np.float32)
    W2T[0::2, 0::2] = MT
    W2T[1::2, 1::2] = MT
    return W2T.astype(BF16)


def kernel(x, group_tensor, group_tensor_inv, Wf):
    nc = _build_nc()
    W2T = _fuse_weights(group_tensor, group_tensor_inv, Wf)
    x = np.asarray(x, np.float32)

    # partition p = channel p//2, pixel-half p%2: just a reshape of (C, HW)
    xr = x.reshape(B, 128, HALF).astype(BF16)
    in_maps = []
    for b in range(B):
        xw = np.empty((128, 128 + HALF), BF16)
        xw[:, 0:128] = W2T
        xw[:, 128:] = xr[b]
        in_maps.append({"xw": xw})

    res = run_bass_kernel_spmd(
        nc, in_maps, core_ids=list(range(N_CORES)), trace=TRACE
    )
    if TRACE:
        kernel.last_results = res
    y = np.stack(
        [
            res.results[b]["y"].astype(np.float32).reshape(COUT, H, W_SP)
            for b in range(B)
        ]
    )
    return y
